# revision 5
# baseline (speedup 1.0000x reference)
"""GNN multi-hop message passing + hop attention on 8 Trainium2 NeuronCores.

Strategy: shard destination nodes across cores (12500 dst nodes each); each
core processes the edges whose dst falls in its range.  Edges are sorted by
128-dst-node "window"; per window the segment-sum over edges becomes a chain
of PE matmuls  aggT[f,d] += msgs[e,f].T-contract-S'[e,d]  where S'[e,d] =
edge_w[e] * (dst_col[e] == d) is built in one fused DVE tensor_scalar per
128-edge tile.  Messages are gathered from a bf16 copy of x with GPSIMD
indirect DMA (int32 row indices).  The hop attention tail is computed per
window in f32.  No cross-core communication is needed.
"""
import sys

sys.path.insert(0, "/opt/trn_rl_repo")

import numpy as np
import ml_dtypes

import concourse.bass as bass
import concourse.bacc as bacc
import concourse.tile as tile
from concourse import mybir
from concourse.bass import IndirectOffsetOnAxis
from concourse.bass_utils import run_bass_kernel_spmd

BF16 = ml_dtypes.bfloat16
NCORES = 8
WIN = 128  # dst nodes per window
NEG_SLOPE = 0.01


def _preprocess(x, edge_src, edge_dst, edge_w, n_nodes, npc, ncores):
    """Bucket edges per (core, hop, window); pad each bucket to a multiple of
    128 edges.  Tile counts are maxed over cores so all cores share one
    program.  Returns (per-core arrays, tiles[k][w], x_bf16)."""
    K = edge_src.shape[0]
    nwin = (npc + WIN - 1) // WIN
    x_bf = np.ascontiguousarray(x.astype(BF16))

    # per (core, hop): sorted edge arrays + per-window counts
    percore = [[None] * K for _ in range(ncores)]
    counts = np.zeros((ncores, K, nwin), np.int64)
    for k in range(K):
        dst = edge_dst[k]
        core = dst // npc
        dl = dst - core * npc
        win = dl // WIN
        col = dl - win * WIN
        for c in range(ncores):
            m = core == c
            w_c = win[m]
            order = np.argsort(w_c, kind="stable")
            percore[c][k] = (
                edge_src[k][m][order].astype(np.int32),
                col[m][order].astype(np.float32),
                edge_w[k][m][order].astype(np.float32),
                w_c[order],
            )
            counts[c, k] = np.bincount(w_c, minlength=nwin)

    tiles = np.maximum(1, (counts.max(axis=0) + 127) // 128)  # [K][nwin]
    sumT = tiles.sum(axis=1)  # [K]

    in_maps = []
    for c in range(ncores):
        m = {"xbf": x_bf}
        for k in range(K):
            src, col, wgt, win_sorted = percore[c][k]
            st = int(sumT[k])
            gidx = np.zeros(st * 128, np.int32)
            cv = np.zeros(st * 128, np.float32)
            wv = np.zeros(st * 128, np.float32)
            off = 0
            pos = 0
            for w in range(tiles.shape[1]):
                n = int(counts[c, k, w])
                sl = slice(off * 128, off * 128 + n)
                gidx[sl] = src[pos : pos + n]
                cv[sl] = col[pos : pos + n]
                wv[sl] = wgt[pos : pos + n]
                pos += n
                off += int(tiles[k, w])
            m[f"gidx{k}"] = np.ascontiguousarray(gidx.reshape(st, 128).T)
            m[f"cv{k}"] = np.ascontiguousarray(cv.reshape(st, 128).T)
            m[f"wv{k}"] = np.ascontiguousarray(wv.reshape(st, 128).T)
        in_maps.append(m)
    return in_maps, tiles, x_bf


def _build(n_nodes, npc, tiles, W_np, attw_np, ncores):
    """Build the Bass program (shared by all cores)."""
    K, nwin = tiles.shape
    sumT = tiles.sum(axis=1)
    f32 = mybir.dt.float32
    bf16 = mybir.dt.bfloat16

    nc = bacc.Bacc("TRN2", target_bir_lowering=False, debug=False,
                   num_devices=ncores)
    xbf = nc.dram_tensor("xbf", [n_nodes, 128], bf16, kind="ExternalInput")
    gidx_d = [nc.dram_tensor(f"gidx{k}", [128, int(sumT[k])], mybir.dt.int32,
                             kind="ExternalInput") for k in range(K)]
    cv_d = [nc.dram_tensor(f"cv{k}", [128, int(sumT[k])], f32,
                           kind="ExternalInput") for k in range(K)]
    wv_d = [nc.dram_tensor(f"wv{k}", [128, int(sumT[k])], f32,
                           kind="ExternalInput") for k in range(K)]
    W_d = nc.dram_tensor("W", [128, 128], f32, kind="ExternalInput")
    aw_d = nc.dram_tensor("attw_rep", [128, 128], f32, kind="ExternalInput")
    iota_d = nc.dram_tensor("iota", [128, 128], bf16, kind="ExternalInput")
    out_d = nc.dram_tensor("out", [npc, 128], f32, kind="ExternalOutput")

    with tile.TileContext(nc) as tc:
        with (
            tc.tile_pool(name="const", bufs=1) as cpool,
            tc.tile_pool(name="meta", bufs=1) as mpool,
            tc.tile_pool(name="msgs", bufs=3) as gpool,
            tc.tile_pool(name="sp", bufs=3) as spool,
            tc.tile_pool(name="work", bufs=3) as wpool,
            tc.tile_pool(name="hops", bufs=2) as hpool,
            tc.tile_pool(name="psA", bufs=2, space="PSUM") as psA,
            tc.tile_pool(name="psH", bufs=2, space="PSUM") as psH,
        ):
            W_sb = cpool.tile([128, 128], f32, tag="W")
            nc.sync.dma_start(W_sb[:], W_d[:])
            aw_sb = cpool.tile([128, 128], f32, tag="aw")
            nc.sync.dma_start(aw_sb[:], aw_d[:])
            iota_sb = cpool.tile([128, 128], bf16, tag="iota")
            nc.sync.dma_start(iota_sb[:], iota_d[:])

            idx_sb, cv_sb, wv_sb = [], [], []
            for k in range(K):
                t_i = mpool.tile([128, int(sumT[k])], mybir.dt.int32, tag=f"idx{k}")
                nc.sync.dma_start(t_i[:], gidx_d[k][:])
                t_c = mpool.tile([128, int(sumT[k])], f32, tag=f"cv{k}")
                nc.sync.dma_start(t_c[:], cv_d[k][:])
                t_w = mpool.tile([128, int(sumT[k])], f32, tag=f"wv{k}")
                nc.sync.dma_start(t_w[:], wv_d[k][:])
                idx_sb.append(t_i)
                cv_sb.append(t_c)
                wv_sb.append(t_w)

            offs = np.zeros((K, nwin), np.int64)
            for k in range(K):
                offs[k, 1:] = np.cumsum(tiles[k])[:-1]

            for w in range(nwin):
                rows = min(WIN, npc - w * WIN)
                h_sb = []
                s3 = wpool.tile([128, 4], f32, tag="s3")
                for k in range(K):
                    T = int(tiles[k, w])
                    off = int(offs[k, w])
                    msgs = gpool.tile([128, T, 128], bf16, tag="msgs")
                    for t in range(T):
                        nc.gpsimd.indirect_dma_start(
                            out=msgs[:, t, :],
                            out_offset=None,
                            in_=xbf[:],
                            in_offset=IndirectOffsetOnAxis(
                                ap=idx_sb[k][:, off + t : off + t + 1], axis=0),
                        )
                    sp = spool.tile([128, T * 128], bf16, tag="sp")
                    for t in range(T):
                        nc.vector.tensor_scalar(
                            out=sp[:, t * 128 : (t + 1) * 128],
                            in0=iota_sb[:],
                            scalar1=cv_sb[k][:, off + t : off + t + 1],
                            scalar2=wv_sb[k][:, off + t : off + t + 1],
                            op0=mybir.AluOpType.is_equal,
                            op1=mybir.AluOpType.mult,
                        )
                    aggT = psA.tile([128, 128], f32, tag="aggT")
                    for t in range(T):
                        nc.tensor.matmul(
                            out=aggT[:],
                            lhsT=msgs[:, t, :],
                            rhs=sp[:, t * 128 : (t + 1) * 128],
                            start=(t == 0),
                            stop=(t == T - 1),
                        )
                    aggT_sb = wpool.tile([128, 128], f32, tag="aggTsb")
                    nc.scalar.activation(aggT_sb[:], aggT[:],
                                         mybir.ActivationFunctionType.Copy)
                    h_ps = psH.tile([128, 128], f32, tag="h")
                    nc.tensor.matmul(out=h_ps[:], lhsT=aggT_sb[:], rhs=W_sb[:],
                                     start=True, stop=True)
                    hk_s = wpool.tile([128, 128], f32, tag="hks")
                    nc.scalar.activation(hk_s[:], h_ps[:],
                                         mybir.ActivationFunctionType.Copy,
                                         scale=NEG_SLOPE)
                    hk = hpool.tile([128, 128], f32, tag=f"h{k}")
                    nc.vector.tensor_tensor(out=hk[:], in0=hk_s[:], in1=h_ps[:],
                                            op=mybir.AluOpType.max)
                    h_sb.append(hk)
                    tmp = wpool.tile([128, 128], f32, tag="tmp")
                    nc.vector.tensor_tensor(out=tmp[:], in0=hk[:], in1=aw_sb[:],
                                            op=mybir.AluOpType.mult)
                    nc.vector.tensor_reduce(
                        out=s3[:, k : k + 1], in_=tmp[:],
                        axis=mybir.AxisListType.X, op=mybir.AluOpType.add)

                # hop softmax + weighted sum (f32)
                negm = wpool.tile([128, 1], f32, tag="negm")
                nc.vector.tensor_reduce(out=negm[:], in_=s3[:, :K],
                                        axis=mybir.AxisListType.X,
                                        op=mybir.AluOpType.max, negate=True)
                e3 = wpool.tile([128, 4], f32, tag="e3")
                ssum = wpool.tile([128, 1], f32, tag="ssum")
                nc.scalar.activation(e3[:, :K], s3[:, :K],
                                     mybir.ActivationFunctionType.Exp,
                                     bias=negm[:, 0:1], accum_out=ssum[:])
                rcp = wpool.tile([128, 1], f32, tag="rcp")
                nc.vector.reciprocal(rcp[:], ssum[:])
                acc = wpool.tile([128, 128], f32, tag="acc")
                nc.vector.tensor_scalar(out=acc[:], in0=h_sb[0][:],
                                        scalar1=e3[:, 0:1], scalar2=None,
                                        op0=mybir.AluOpType.mult)
                for k in range(1, K):
                    t2 = wpool.tile([128, 128], f32, tag="acc2")
                    nc.vector.tensor_scalar(out=t2[:], in0=h_sb[k][:],
                                            scalar1=e3[:, k : k + 1],
                                            scalar2=None,
                                            op0=mybir.AluOpType.mult)
                    nc.vector.tensor_tensor(out=acc[:], in0=acc[:], in1=t2[:],
                                            op=mybir.AluOpType.add)
                outt = wpool.tile([128, 128], f32, tag="outt")
                nc.vector.tensor_scalar(out=outt[:], in0=acc[:],
                                        scalar1=rcp[:, 0:1], scalar2=None,
                                        op0=mybir.AluOpType.mult)
                nc.sync.dma_start(out_d[w * WIN : w * WIN + rows, :],
                                  outt[:rows, :])
    nc.compile()
    return nc


def kernel(x, edge_src, edge_dst, edge_w, W, att_w, att_b):
    x = np.asarray(x, np.float32)
    edge_src = np.asarray(edge_src, np.int32)
    edge_dst = np.asarray(edge_dst, np.int32)
    edge_w = np.asarray(edge_w, np.float32)
    W = np.asarray(W, np.float32)
    att_w = np.asarray(att_w, np.float32)
    att_b = np.asarray(att_b, np.float32)

    n_nodes = x.shape[0]
    npc = n_nodes // NCORES
    in_maps, tiles, _ = _preprocess(x, edge_src, edge_dst, edge_w,
                                    n_nodes, npc, NCORES)
    attw_rep = np.ascontiguousarray(
        np.broadcast_to(att_w[:, 0][None, :], (128, 128)).astype(np.float32))
    iota = np.ascontiguousarray(
        np.broadcast_to(np.arange(128, dtype=np.float32)[None, :],
                        (128, 128)).astype(BF16))
    for m in in_maps:
        m["W"] = W
        m["attw_rep"] = attw_rep
        m["iota"] = iota

    nc = _build(n_nodes, npc, tiles, W, att_w, NCORES)
    res = run_bass_kernel_spmd(nc, in_maps, list(range(NCORES)))
    out = np.concatenate([res.results[c]["out"] for c in range(NCORES)], axis=0)
    # att_b shifts all hop scores equally -> cancels in softmax; output is
    # independent of it (and it is zeros in the reference inputs).
    return out.astype(np.float32)


if __name__ == "__main__":
    rng = np.random.default_rng(0)
    N, E, K = 100_000, 1_600_000, 3
    ins = {
        "x": rng.standard_normal((N, 128)).astype(np.float32),
        "edge_src": rng.integers(0, N, (K, E)).astype(np.int32),
        "edge_dst": rng.integers(0, N, (K, E)).astype(np.int32),
        "edge_w": rng.random((K, E)).astype(np.float32),
        "W": (rng.standard_normal((128, 128)) / np.sqrt(128)).astype(np.float32),
        "att_w": (rng.standard_normal((128, 1)) / np.sqrt(128)).astype(np.float32),
        "att_b": np.zeros(1, np.float32),
    }
    out = kernel(**ins)
    print("out", out.shape, out.dtype)
